# revision 1
# baseline (speedup 1.0000x reference)
"""Bass/Tile TRN2 kernel for nn_CausalAttention (softmax + tril-matmul renorm).

Math restructuring (per core, row block of B = SEQ/n_cores rows):
    q = x @ wq ; k = x @ wk ; v = x @ wv
    z = q @ k.T / sqrt(D) ;  s = exp(z)            (softmax norm cancels below)
    masked[i,j] = sum_{t>=j} s[i,t]                (suffix sum == s @ tril)
    out = (masked @ v) / rowsum(masked)

All work happens in transposed layouts ([feature/key on partitions, query on free]):
    sT[t,i] = s[i,t], computed as zT = KT-chunks.T @ qT per 128-key tile.
    masked0T_r = tril128 @ sT_r          (within-tile suffix sums, one matmul per tile)
    CS[r,i]  = colsum(sT_r)              (selector matmuls into one PSUM tile)
    SUF[r,i] = sum_{r'>r} CS[r',i]       (one strict-tril matmul)
    numT = sum_r V_r.T-mm(masked0T_r) + VS.T-mm(SUF)     where VS[r,:] = colsum(V_r)
    den[i]  = sum_t (t+1) s[t,i]         (weight-column matmuls)
    out = (numT / den).T

Projections (q/K/V row blocks, VS) are computed locally per core in bf16; K is
exchanged with an early AllGather (hidden under q/V projections), V+VS with a
second AllGather (hidden under the scores phase).
"""
import numpy as np
from contextlib import ExitStack

import concourse.bass as bass
import concourse.tile as tile
from concourse import bacc, mybir

F32 = mybir.dt.float32
BF16 = mybir.dt.bfloat16
FP8 = mybir.dt.float8e4
U8 = mybir.dt.uint8
AX = mybir.AxisListType
AF = mybir.ActivationFunctionType
ALU = mybir.AluOpType

P = 128


def make_consts(T):
    tril = np.tril(np.ones((P, P), np.float32))  # [t, j] = 1 if t >= j
    stril = np.tri(T, T, -1, dtype=np.float32)  # [r', r] = 1 if r' > r
    # block r ([P, 33]): col m<T is [m==r] (tile colsum selector); col 32 is
    # the den weight column (t+1) for tile r (pinned to partition 32, a legal
    # engine partition base for the later row extraction).
    MB = 33
    seld = np.zeros((P, T * MB), np.float32)
    for rr in range(T):
        seld[:, rr * MB + rr] = 1.0
        seld[:, rr * MB + 32] = P * rr + np.arange(P) + 1.0
    ident = np.eye(P, dtype=np.float32)
    import ml_dtypes
    bf = lambda a: a.astype(ml_dtypes.bfloat16)
    return dict(c_tril=bf(tril), c_stril=bf(stril), c_seld=bf(seld),
                c_ident=ident, c_identbf=bf(ident))


def build(SEQ=4096, D=1024, n_cores=8):
    T = SEQ // P           # global 128-key tiles
    TL = T // n_cores      # local tiles per core
    B = P * TL             # rows per core
    DC = D // P            # feature chunks
    W = min(512, D)        # moving free width for D-wide matmuls
    NH = D // W
    assert B <= 512 and T <= P and D % W == 0 and SEQ % (P * n_cores) == 0
    scale = float(1.0 / np.sqrt(D) / 64.0)   # wq,wk each prescaled x8

    nc = bacc.Bacc("TRN2", target_bir_lowering=False, debug=False, num_devices=n_cores)

    x = nc.dram_tensor("x", [B, D], BF16, kind="ExternalInput")
    wq_d = nc.dram_tensor("wq", [D, D], FP8, kind="ExternalInput")
    wk_d = nc.dram_tensor("wk", [D, D], FP8, kind="ExternalInput")
    wv_d = nc.dram_tensor("wv", [D, D], BF16, kind="ExternalInput")
    c_tril = nc.dram_tensor("c_tril", [P, P], BF16, kind="ExternalInput")
    c_stril = nc.dram_tensor("c_stril", [T, T], BF16, kind="ExternalInput")
    MB = 33
    c_seld = nc.dram_tensor("c_seld", [P, T * MB], BF16, kind="ExternalInput")
    c_ident = nc.dram_tensor("c_ident", [P, P], F32, kind="ExternalInput")
    c_identbf = nc.dram_tensor("c_identbf", [P, P], BF16, kind="ExternalInput")
    out = nc.dram_tensor("out", [B, D], F32, kind="ExternalOutput")

    KTN = D * B            # cc1: KT region [D, B] row-major (bf16)
    VNB = B * D            # cc2 bytes: V region [B, D] fp8e4
    XRB = 2 * D * TL       # cc2 bytes: xrs region [D, TL] bf16
    CC2N = VNB + XRB

    with tile.TileContext(nc) as tc, ExitStack() as top:
        dram = top.enter_context(tc.tile_pool(name="dram", bufs=1, space="DRAM"))
        NHALF = 1
        B2 = B // NHALF
        KH = D * B2
        cc1a_in = dram.tile([KH], FP8)
        cc1a_out = dram.tile([n_cores, KH], FP8, addr_space="Shared")
        if NHALF == 2:
            cc1b_in = dram.tile([KH], FP8)
            cc1b_out = dram.tile([n_cores, KH], FP8, addr_space="Shared")
        else:
            cc1b_in, cc1b_out = cc1a_in, cc1a_out
        cc2_in = dram.tile([CC2N], U8)
        cc2_out = dram.tile([n_cores, CC2N], U8, addr_space="Shared")

        consts = top.enter_context(tc.tile_pool(name="consts", bufs=1))
        identbf = consts.tile([P, P], BF16)
        nc.sync.dma_start(identbf[:], c_identbf.ap())
        tril_sb = consts.tile([P, P], BF16)
        nc.sync.dma_start(tril_sb[:], c_tril.ap())
        stril_sb = consts.tile([T, T], BF16)
        nc.sync.dma_start(stril_sb[:], c_stril.ap())
        seld_sb = consts.tile([P, T * MB], BF16)
        nc.gpsimd.dma_start(seld_sb[:], c_seld.ap())
        ident_sb = consts.tile([P, P], F32)
        nc.gpsimd.dma_start(ident_sb[:], c_ident.ap())

        persist = top.enter_context(tc.tile_pool(name="persist", bufs=1))
        qT = persist.tile([P, DC * B], FP8)          # q.T row block
        vs_sb = persist.tile([T, D], BF16)           # per-tile V colsums
        xrs_g = persist.tile([P, DC * T], BF16)      # gathered per-tile x row sums
        suf_sb = persist.tile([T, B], BF16)
        cs_sb = persist.tile([T, B], BF16)
        recip = persist.tile([P, TL], F32)
        dennat = persist.tile([P, TL], F32)
        den_pad = persist.tile([P, B], F32)

        wv_sb = persist.tile([P, DC * D], BF16)

        # ------------------- stage 1: local projections (bf16) -------------------
        with ExitStack() as s1:
            xpool = s1.enter_context(tc.tile_pool(name="xload", bufs=6))
            xTp = s1.enter_context(tc.tile_pool(name="xT", bufs=1))
            xT = xTp.tile([P, DC * B], BF16)
            xT8 = xTp.tile([P, DC * B], FP8)

            wpool = s1.enter_context(tc.tile_pool(name="w", bufs=1))
            wk_sb = wpool.tile([P, DC * D], FP8)
            wq_sb = wpool.tile([P, DC * D], FP8)

            trps = s1.enter_context(tc.tile_pool(name="trps", bufs=3, space="PSUM"))
            # x.T via matmul with identity: out = x_tile.T @ I (bf16 in, f32 psum).
            # All TL row-tiles of one d-chunk share a PSUM tile -> one wide copy.
            xts = []
            for tcc in range(TL):
                xt_ = xpool.tile([P, D], BF16, tag="x", name=f"xt_{tcc}")
                nc.sync.dma_start(xt_[:], x.ap()[tcc * P:(tcc + 1) * P, :])
                xts.append(xt_)
            for dc in range(DC):
                ps = trps.tile([P, B], F32, tag="tr")
                for tcc in range(TL):
                    nc.tensor.matmul(ps[:, tcc * P:(tcc + 1) * P],
                                     xts[tcc][:, dc * P:(dc + 1) * P], identbf[:],
                                     start=True, stop=True)
                (nc.vector.tensor_copy if dc % 2 == 0 else nc.scalar.copy)(
                    xT[:, dc * B:(dc + 1) * B], ps[:])
                (nc.scalar.copy if dc % 2 == 0 else nc.vector.tensor_copy)(
                    xT8[:, dc * B:(dc + 1) * B], ps[:])

            # weights (already bf16 from host); wk first: the K path gates cc1
            for dc in range(DC):
                nc.sync.dma_start(wk_sb[:, dc * D:(dc + 1) * D], wk_d.ap()[dc * P:(dc + 1) * P, :])
            for dc in range(DC):
                nc.gpsimd.dma_start(wq_sb[:, dc * D:(dc + 1) * D], wq_d.ap()[dc * P:(dc + 1) * P, :])
            for dc in range(DC):
                nc.gpsimd.dma_start(wv_sb[:, dc * D:(dc + 1) * D], wv_d.ap()[dc * P:(dc + 1) * P, :])

            pps = s1.enter_context(tc.tile_pool(name="pps", bufs=4, space="PSUM"))
            kvl = s1.enter_context(tc.tile_pool(name="kvl", bufs=1))
            kT_loc = kvl.tile([P, DC * B], FP8)

            # kT projection -> cc1 (gates the first collective)
            wk3 = wk_sb.rearrange("p (dc d) -> p dc d", dc=DC)
            wq3 = wq_sb.rearrange("p (dc d) -> p dc d", dc=DC)
            xT83 = xT8.rearrange("p (dc b) -> p dc b", dc=DC)
            for dco in range(DC):
                k_ps = pps.tile([P, B], F32, tag="pp", name="k_ps")
                for pp_ in range(DC // 2):
                    nc.tensor.matmul(
                        k_ps[:],
                        wk3[:, 2 * pp_:2 * pp_ + 2, dco * P:(dco + 1) * P],
                        xT83[:, 2 * pp_:2 * pp_ + 2, :],
                        start=(pp_ == 0), stop=(pp_ == DC // 2 - 1),
                        perf_mode=mybir.MatmulPerfMode.DoubleRow,
                    )
                nc.vector.tensor_copy(kT_loc[:, dco * B:(dco + 1) * B], k_ps[:])
                for h, cc_h in enumerate([cc1a_in, cc1b_in][:NHALF]):
                    nc.sync.dma_start(
                        cc_h[dco * P * B2:(dco + 1) * P * B2].rearrange("(p i) -> p i", p=P),
                        kT_loc[:, dco * B + h * B2: dco * B + (h + 1) * B2],
                    )

            # K gather, split by key halves: phase A starts on half a as soon
            # as it lands, while half b is still on the wire
            nc.gpsimd.collective_compute(
                "AllGather", ALU.bypass,
                replica_groups=[list(range(n_cores))],
                ins=[cc1a_in.opt()], outs=[cc1a_out.opt()],
            )
            if NHALF == 2:
                nc.gpsimd.collective_compute(
                    "AllGather", ALU.bypass,
                    replica_groups=[list(range(n_cores))],
                    ins=[cc1b_in.opt()], outs=[cc1b_out.opt()],
                )

            # qT projection
            for dco in range(DC):
                q_ps = pps.tile([P, B], F32, tag="pp", name="q_ps")
                for pp_ in range(DC // 2):
                    nc.tensor.matmul(
                        q_ps[:],
                        wq3[:, 2 * pp_:2 * pp_ + 2, dco * P:(dco + 1) * P],
                        xT83[:, 2 * pp_:2 * pp_ + 2, :],
                        start=(pp_ == 0), stop=(pp_ == DC // 2 - 1),
                        perf_mode=mybir.MatmulPerfMode.DoubleRow,
                    )
                nc.vector.tensor_copy(qT[:, dco * B:(dco + 1) * B], q_ps[:])

            # V row block (natural layout, fp8e4) -> cc2
            for tcc in range(TL):
                vl = xpool.tile([P, D], FP8, tag="vl")
                for nh in range(NH):
                    v_ps = pps.tile([P, W], F32, tag="pp", name="v_ps")
                    for dci in range(DC):
                        nc.tensor.matmul(
                            v_ps[:],
                            xT[:, dci * B + tcc * P: dci * B + (tcc + 1) * P],
                            wv_sb[:, dci * D + nh * W: dci * D + (nh + 1) * W],
                            start=(dci == 0), stop=(dci == DC - 1),
                        )
                    nc.vector.tensor_copy(vl[:, nh * W:(nh + 1) * W], v_ps[:])
                nc.sync.dma_start(
                    cc2_in[tcc * P * D:(tcc + 1) * P * D].rearrange("(p d) -> p d", p=P),
                    vl[:].bitcast(U8),
                )

            # per-tile x row sums (bf16) -> cc2; VS recomputed after the gather
            xrs_f = wpool.tile([P, DC * TL], F32)
            xrs_bf = wpool.tile([P, DC * TL], BF16)
            for dc in range(DC):
                nc.vector.reduce_sum(
                    xrs_f[:, dc * TL:(dc + 1) * TL],
                    xT[:, dc * B:(dc + 1) * B].rearrange("p (t i) -> p t i", t=TL),
                    axis=AX.X,
                )
            nc.vector.tensor_copy(xrs_bf[:], xrs_f[:])
            for dc in range(DC):
                nc.sync.dma_start(
                    cc2_in[VNB + dc * P * TL * 2: VNB + (dc + 1) * P * TL * 2]
                    .rearrange("(p t) -> p t", p=P),
                    xrs_bf[:, dc * TL:(dc + 1) * TL].bitcast(U8),
                )

        # second collective: gather V + VS (runs under phase A)
        nc.gpsimd.collective_compute(
            "AllGather", ALU.bypass,
            replica_groups=[list(range(n_cores))],
            ins=[cc2_in.opt()], outs=[cc2_out.opt()],
        )

        m0p = top.enter_context(tc.tile_pool(name="m0", bufs=1))
        m0 = m0p.tile([P, T * B], FP8)   # masked0T (within-tile suffix sums)

        # ------------------- phase A: scores / exp / per-tile sums -------------------
        with ExitStack() as pa:
            ktp = pa.enter_context(tc.tile_pool(name="kt", bufs=4))
            stp = pa.enter_context(tc.tile_pool(name="st", bufs=4))
            ztp = pa.enter_context(tc.tile_pool(name="zt", bufs=3, space="PSUM"))
            mtp = pa.enter_context(tc.tile_pool(name="mt", bufs=3, space="PSUM"))
            csp = pa.enter_context(tc.tile_pool(name="csp", bufs=1, space="PSUM"))
            cs_ps = csp.tile([MB, B], F32)

            TLH = TL // NHALF
            first = True
            for half in range(NHALF):
                cc_h = [cc1a_out, cc1b_out][half]
                for rc in range(n_cores):
                    ktc = ktp.tile([P, DC * B2], FP8, tag="kt")
                    for dc in range(DC):
                        nc.sync.dma_start(
                            ktc[:, dc * B2:(dc + 1) * B2],
                            cc_h[rc, dc * P * B2:(dc + 1) * P * B2].rearrange("(p i) -> p i", p=P),
                        )
                    ktc3 = ktc.rearrange("p (dc i) -> p dc i", dc=DC)
                    qT3 = qT.rearrange("p (dc b) -> p dc b", dc=DC)
                    for sub2 in range(TLH):
                        sub = half * TLH + sub2
                        rg = rc * TL + sub
                        last = (half == NHALF - 1) and (rc == n_cores - 1) and (sub2 == TLH - 1)
                        zt = ztp.tile([P, B], F32, tag="zt")
                        for pp in range(DC // 2):
                            nc.tensor.matmul(
                                zt[:],
                                ktc3[:, 2 * pp:2 * pp + 2, sub2 * P:(sub2 + 1) * P],
                                qT3[:, 2 * pp:2 * pp + 2, :],
                                start=(pp == 0), stop=(pp == DC // 2 - 1),
                                perf_mode=mybir.MatmulPerfMode.DoubleRow,
                            )
                        st = stp.tile([P, B], BF16, tag="st")
                        nc.scalar.activation(st[:], zt[:], AF.Exp, scale=scale)
                        mt = mtp.tile([P, B], F32, tag="mt")
                        nc.tensor.matmul(mt[:], tril_sb[:], st[:], start=True, stop=True)
                        nc.tensor.matmul(
                            cs_ps[:], seld_sb[:, rg * MB:(rg + 1) * MB], st[:],
                            start=first, stop=last,
                        )
                        first = False
                        nc.vector.tensor_scalar(
                            m0[:, rg * B:(rg + 1) * B], mt[:], 0.0625, None, op0=ALU.mult)

            nc.vector.tensor_copy(cs_sb[:], cs_ps[0:T, :])
            nc.vector.memset(den_pad[:], 0.0)
            nc.vector.tensor_copy(den_pad[32:33, :], cs_ps[32:33, :])
            suf_ps = mtp.tile([T, B], F32, tag="mt")
            nc.tensor.matmul(suf_ps[:], stril_sb[:], cs_sb[:], start=True, stop=True)
            nc.vector.tensor_scalar(suf_sb[:], suf_ps[:], 0.0625, None, op0=ALU.mult)

        # gathered x row sums -> VS = xrs.T-mm(wv)  [T, D]
        for dc in range(DC):
            nc.sync.dma_start(
                xrs_g[:, dc * T:(dc + 1) * T].bitcast(U8),
                cc2_out[0:n_cores, VNB + dc * P * TL * 2: VNB + (dc + 1) * P * TL * 2]
                .rearrange("c (p t) -> p c t", p=P),
            )
        with ExitStack() as svs:
            vsps = svs.enter_context(tc.tile_pool(name="vsps", bufs=2, space="PSUM"))
            for nh in range(NH):
                vs_ps = vsps.tile([T, W], F32, tag="vs")
                for dci in range(DC):
                    nc.tensor.matmul(
                        vs_ps[:],
                        xrs_g[:, dci * T:(dci + 1) * T],
                        wv_sb[:, dci * D + nh * W: dci * D + (nh + 1) * W],
                        start=(dci == 0), stop=(dci == DC - 1),
                    )
                nc.vector.tensor_copy(vs_sb[:, nh * W:(nh + 1) * W], vs_ps[:])


        # ------------------- phase B: numT accumulation (two d'-groups) -------------------
        H = DC // 2
        with ExitStack() as pb:
            trp2 = pb.enter_context(tc.tile_pool(name="trp2", bufs=4, space="PSUM"))
            vrp = pb.enter_context(tc.tile_pool(name="vr", bufs=4))
            outp = pb.enter_context(tc.tile_pool(name="outp", bufs=4))
            nump = pb.enter_context(tc.tile_pool(name="nump", bufs=H, space="PSUM"))
            nsbp = pb.enter_context(tc.tile_pool(name="nsb", bufs=H))

            # 1/den (overlaps the first group's matmuls)
            for sub in range(TL):
                dps = trp2.tile([P, P], F32, tag="tr2")
                nc.tensor.transpose(dps[:], den_pad[:, sub * P:(sub + 1) * P], ident_sb[:])
                nc.vector.tensor_copy(dennat[:, sub:sub + 1], dps[:, 32:33])
            nc.vector.reciprocal(recip[:], dennat[:])

            D2 = D // 2
            for g in range(2):
                nums = [nump.tile([P, B], F32, tag="num", name=f"num_ps{g}_{i}")
                        for i in range(H)]
                # fp8 DoubleRow: two 128-key tiles per matmul (contraction 256).
                # V-pairs OPEN the psum group (only need m0 + the V gather) and
                # the VS x SUF term CLOSES it (SUF is ready only after all of
                # phase A) -- lets phase B overlap phase A's tail.
                if TL % 2 == 0:
                    for rc in range(n_cores):
                        for pr in range(TL // 2):
                            rg = rc * TL + 2 * pr
                            vp = vrp.tile([P, 2 * D2], FP8, tag="vr")
                            for t_ in range(2):
                                nc.sync.dma_start(
                                    vp[:, t_ * D2:(t_ + 1) * D2].bitcast(U8),
                                    cc2_out[rc, (2 * pr + t_) * P * D:(2 * pr + t_ + 1) * P * D]
                                    .rearrange("(p d) -> p d", p=P)[:, g * D2:(g + 1) * D2],
                                )
                            vp3 = vp.rearrange("p (two n) -> p two n", two=2)
                            m3 = m0[:, rg * B:(rg + 2) * B].rearrange("p (two b) -> p two b", two=2)
                            for i in range(H):
                                nc.tensor.matmul(
                                    nums[i][:], vp3[:, :, i * P:(i + 1) * P], m3,
                                    start=(rg == 0), stop=False,
                                    perf_mode=mybir.MatmulPerfMode.DoubleRow,
                                )
                else:
                    for rc in range(n_cores):
                        for sub in range(TL):
                            rg = rc * TL + sub
                            vp = vrp.tile([P, D2], FP8, tag="vr")
                            nc.sync.dma_start(
                                vp[:].bitcast(U8),
                                cc2_out[rc, sub * P * D:(sub + 1) * P * D]
                                .rearrange("(p d) -> p d", p=P)[:, g * D2:(g + 1) * D2],
                            )
                            for i in range(H):
                                nc.tensor.matmul(
                                    nums[i][:], vp[:, i * P:(i + 1) * P],
                                    m0[:, rg * B:(rg + 1) * B],
                                    start=(rg == 0), stop=False,
                                )
                for i in range(H):
                    dc2 = g * H + i
                    nc.tensor.matmul(
                        nums[i][:], vs_sb[:, dc2 * P:(dc2 + 1) * P], suf_sb[:],
                        start=False, stop=True,
                    )
                # group epilogue: copy out of PSUM, transpose back, scale, store
                num_sb = []
                for i in range(H):
                    t_ = nsbp.tile([P, B], F32, tag="nsb", name=f"num_sb{g}_{i}")
                    nc.vector.tensor_copy(t_[:], nums[i][:])
                    num_sb.append(t_)
                for sub in range(TL):
                    ot = outp.tile([P, D // 2], F32, tag="ot")
                    for i in range(H):
                        tps = trp2.tile([P, P], F32, tag="tr2")
                        nc.tensor.transpose(tps[:], num_sb[i][:, sub * P:(sub + 1) * P], ident_sb[:])
                        nc.vector.tensor_scalar(
                            ot[:, i * P:(i + 1) * P], tps[:], recip[:, sub:sub + 1], 16.0,
                            op0=ALU.mult, op1=ALU.mult,
                        )
                    nc.sync.dma_start(
                        out.ap()[sub * P:(sub + 1) * P, g * (D // 2):(g + 1) * (D // 2)],
                        ot[:],
                    )

    nc.compile()
    return nc


def make_in_maps(x_full, wq, wk, wv, n_cores=8):
    import ml_dtypes
    bf = lambda a: np.ascontiguousarray(a).astype(ml_dtypes.bfloat16)
    f8 = lambda a: np.ascontiguousarray(a).astype(ml_dtypes.float8_e4m3)
    SEQ, D = x_full.shape
    T = SEQ // P
    B = SEQ // n_cores
    consts = make_consts(T)
    # wq/wk prescaled x8 into fp8e4 (folded back out in the exp scale)
    wq8, wk8, wvb = f8(wq * 8.0), f8(wk * 8.0), bf(wv)
    in_maps = []
    for c in range(n_cores):
        m = {"x": bf(x_full[c * B:(c + 1) * B]),
             "wq": wq8, "wk": wk8, "wv": wvb}
        m.update(consts)
        in_maps.append(m)
    return in_maps


def algo_ref(x, wq, wk, wv):
    """Numpy float64 reference of the restructured math (for validation)."""
    x = x.astype(np.float64)
    q = x @ wq.astype(np.float64)
    k = x @ wk.astype(np.float64)
    v = x @ wv.astype(np.float64)
    z = q @ k.T / np.sqrt(k.shape[1])
    s = np.exp(z)
    masked = np.cumsum(s[:, ::-1], axis=1)[:, ::-1]
    num = masked @ v
    den = masked.sum(axis=1)
    return (num / den[:, None]).astype(np.float32)



# ----------------------------------------------------------------------------
# Harness entry point: full (unsharded) inputs -> full output.
# ----------------------------------------------------------------------------
SEQ, D_IN, N_CORES = 4096, 1024, 8
_built = {}


def _get_nc(SEQ_=SEQ, D_=D_IN, n_cores=N_CORES):
    key = (SEQ_, D_, n_cores)
    if key not in _built:
        _built[key] = build(SEQ=SEQ_, D=D_, n_cores=n_cores)
    return _built[key]


def run(x, wq, wk, wv, trace=False, **spmd_kwargs):
    from concourse.bass_utils import run_bass_kernel_spmd

    x = np.ascontiguousarray(np.asarray(x, dtype=np.float32))
    wq = np.ascontiguousarray(np.asarray(wq, dtype=np.float32))
    wk = np.ascontiguousarray(np.asarray(wk, dtype=np.float32))
    wv = np.ascontiguousarray(np.asarray(wv, dtype=np.float32))
    n_cores = N_CORES
    nc = _get_nc(x.shape[0], x.shape[1], n_cores)
    in_maps = make_in_maps(x, wq, wk, wv, n_cores=n_cores)
    res = run_bass_kernel_spmd(nc, in_maps, list(range(n_cores)),
                               trace=trace, **spmd_kwargs)
    out = np.concatenate([res.results[c]["out"] for c in range(n_cores)], axis=0)
    return out, res


def kernel(x, wq, wk, wv):
    out, _ = run(x, wq, wk, wv, trace=False)
    return out



# revision 13
# speedup vs baseline: 1.1690x; 1.1690x over previous
"""Bass/Tile TRN2 kernel for nn_CausalAttention (softmax + tril-matmul renorm).

V3 restructure around the identity  masked @ v == s @ cumsum(v):
    out[i] = (sum_t s[i,t] * PV[t]) / (sum_t (t+1) * s[i,t]),   s = exp(q k^T / sqrt(D))
with PV[t] = prefix-sum of v rows, split (for fp8 precision) as
    PV[t] = PVt_within[t] + CVS[tile(t)]
where PVt_within is the within-128-tile prefix (small magnitudes, fp8 ok) and
CVS carries all cross-tile aggregates in bf16 (tile colsums VS computed in
high precision from x row-sums: VS = xrs @ wv_bf).

Per core (512 q rows):
  local:  xT (transposes, bf16+fp8), kT/qT (fp8 DR), v (fp8 DR, full scale),
          PVt = within-tile prefix of v (triu matmuls), xrs row sums,
          VS = xrsT-chunks @ wv_bf (bf16)
  comm:   AllGather kT in two key-halves (scores start on half 1),
          AllGather [PVt fp8 | VS f32]
  A:      zT tiles [key,q] (fp8 DR), exp -> m0 fp8, per-pair selector matmul
          -> rs (per-tile rowsums of s, rows 0..31) + den (row 64)
  B:      num[q,d] = sum_pairs m0_pair^T mm PV_pair (fp8 DR)
          + rank-32 close: rs^T mm CVS (bf16), CVS = stril32 @ VS_all
          out = num * recip(den) / 64
"""
import numpy as np
from contextlib import ExitStack

import concourse.bass as bass
import concourse.tile as tile
from concourse import bacc, mybir

F32 = mybir.dt.float32
BF16 = mybir.dt.bfloat16
FP8 = mybir.dt.float8e4
U8 = mybir.dt.uint8
AX = mybir.AxisListType
AF = mybir.ActivationFunctionType
ALU = mybir.AluOpType
DR = mybir.MatmulPerfMode.DoubleRow

P = 128
EXP_BIAS = -2.0  # s' = exp(z - 2): keeps fp8 m0 in range; cancels in num/den


def make_consts(SEQ, n_cores):
    import ml_dtypes
    bf = lambda a: a.astype(ml_dtypes.bfloat16)
    f8 = lambda a: a.astype(ml_dtypes.float8_e4m3)
    T = SEQ // P
    NPAIR = T // 2
    ident = np.eye(P, dtype=np.float32)
    # PVt stationary: within-tile prefix stat[j, r] = 1 if j <= r  (triu)
    triu = np.triu(np.ones((P, P), np.float32))
    # selector+w [P, NPAIR, 2, 128]: col t (t<T) = [tile == t], col 64 =
    # (t_glob+1)/64.  (full 128 cols: dual-fp8 LdWeights rejects
    # partial-column tiles)
    selw = np.zeros((P, NPAIR, 2, 128), np.float32)
    for p_ in range(NPAIR):
        for s_ in range(2):
            selw[:, p_, s_, 2 * p_ + s_] = 1.0
            t_glob = 256 * p_ + 128 * s_ + np.arange(P)
            selw[:, p_, s_, 64] = (t_glob + 1.0) / 64.0
    # cross-tile strict prefix [t', t] = 1 if t' < t
    stril32T = np.triu(np.ones((T, T), np.float32), 1)
    return dict(
        c_identbf=bf(ident), c_ident=ident,
        c_triu8=f8(triu),
        c_selw=f8(selw.reshape(P, NPAIR * 2 * 128)),
        c_stril32T=bf(stril32T),
    )


def build(SEQ=4096, D=1024, n_cores=8):
    T = SEQ // P           # global 128-key tiles (32)
    TL = T // n_cores      # local tiles per core (4)
    B = P * TL             # rows per core (512)
    B2 = B // 2            # key half per core (256)
    DC = D // P            # feature chunks (8)
    NPAIR = T // 2         # global 256-key pairs (16)
    QC = B // P            # q chunks per core (4)
    assert B == 512 and DC == 8 and TL == 4
    scale = float(1.0 / np.sqrt(D) / 64.0)   # wq,wk each prescaled x8

    nc = bacc.Bacc("TRN2", target_bir_lowering=False, debug=False, num_devices=n_cores)

    x = nc.dram_tensor("x", [B, D], BF16, kind="ExternalInput")
    wq_d = nc.dram_tensor("wq", [P, DC * D], FP8, kind="ExternalInput")
    wk_d = nc.dram_tensor("wk", [P, DC * D], FP8, kind="ExternalInput")
    wv_d = nc.dram_tensor("wv", [P, DC * D], FP8, kind="ExternalInput")
    wvb_d = nc.dram_tensor("wvb", [P, DC * D], BF16, kind="ExternalInput")
    c_identbf = nc.dram_tensor("c_identbf", [P, P], BF16, kind="ExternalInput")
    c_ident = nc.dram_tensor("c_ident", [P, P], F32, kind="ExternalInput")
    c_triu8 = nc.dram_tensor("c_triu8", [P, P], FP8, kind="ExternalInput")
    c_selw = nc.dram_tensor("c_selw", [P, NPAIR * 2 * 128], FP8, kind="ExternalInput")
    c_stril32T = nc.dram_tensor("c_stril32T", [T, T], BF16, kind="ExternalInput")
    out = nc.dram_tensor("out", [B, D], F32, kind="ExternalOutput")

    KH = D * B2                  # one kT key-half, fp8 bytes
    PVN = TL * P * D             # PVt payload, fp8 bytes
    VSB = TL * D * 4             # VS payload, f32 bytes
    CC2N = PVN + VSB

    with tile.TileContext(nc) as tc, ExitStack() as top:
        dram = top.enter_context(tc.tile_pool(name="dram", bufs=1, space="DRAM"))
        cc1a_in = dram.tile([KH], FP8)
        cc1a_out = dram.tile([n_cores, KH], FP8, addr_space="Shared")
        cc1b_in = dram.tile([KH], FP8)
        cc1b_out = dram.tile([n_cores, KH], FP8, addr_space="Shared")
        cc2_in = dram.tile([CC2N], U8)
        cc2_out = dram.tile([n_cores, CC2N], U8, addr_space="Shared")

        consts = top.enter_context(tc.tile_pool(name="consts", bufs=1))
        identbf = consts.tile([P, P], BF16)
        nc.sync.dma_start(identbf[:], c_identbf.ap())
        triu8_sb = consts.tile([P, P], FP8)
        nc.gpsimd.dma_start(triu8_sb[:], c_triu8.ap())
        selw_sb = consts.tile([P, NPAIR * 2 * 128], FP8)
        nc.gpsimd.dma_start(selw_sb[:], c_selw.ap())
        stril32T_sb = consts.tile([T, T], BF16)
        nc.gpsimd.dma_start(stril32T_sb[:], c_stril32T.ap())
        ident_sb = consts.tile([P, P], F32)
        nc.gpsimd.dma_start(ident_sb[:], c_ident.ap())
        expb = consts.tile([P, 1], F32)
        nc.vector.memset(expb[:], EXP_BIAS)

        persist = top.enter_context(tc.tile_pool(name="persist", bufs=1))
        qT = persist.tile([P, DC * B], FP8)          # q.T row block
        m0 = persist.tile([P, NPAIR * 2 * B], FP8)   # exp scores, [key, pair, slot, q]
        pvg = persist.tile([P, T * D], FP8)          # gathered PV tiles [key, tile, d]
        rs_sb = persist.tile([T, B], BF16)           # per-tile rowsums of s
        cvs_sb = persist.tile([T, D], BF16)          # cross-tile prefix colsums
        vs_all = persist.tile([T, D], F32)
        vs_bf = persist.tile([T, D], BF16)
        recip = persist.tile([P, QC], F32)
        dennat = persist.tile([P, QC], F32)
        den_pad = persist.tile([P, B], F32)

        # ------------------- stage 1: local projections -------------------
        with ExitStack() as s1:
            xp = s1.enter_context(tc.tile_pool(name="xload", bufs=1))
            xt_sb = xp.tile([P, TL * D], BF16)       # x rows, [p, tile, d]
            nc.sync.dma_start(
                xt_sb.rearrange("p (t d) -> p t d", t=TL),
                x.ap().rearrange("(t p) d -> p t d", p=P))
            xT8 = xp.tile([P, DC * B], FP8)          # x.T, [d, dc, row]
            xTb = xp.tile([P, DC * B], BF16)         # x.T in bf16 (for xrs)
            wk_sb = xp.tile([P, DC * D], FP8)
            nc.sync.dma_start(wk_sb[:], wk_d.ap())
            wq_sb = xp.tile([P, DC * D], FP8)
            nc.gpsimd.dma_start(wq_sb[:], wq_d.ap())
            wv_sb = xp.tile([P, DC * D], FP8)
            nc.gpsimd.dma_start(wv_sb[:], wv_d.ap())
            wvb_sb = xp.tile([P, DC * D], BF16)
            nc.gpsimd.dma_start(wvb_sb[:], wvb_d.ap())
            kT_loc = xp.tile([P, DC * B], FP8)
            vpair = xp.tile([P, 2 * 2 * D], FP8)     # v tiles [row, pairidx, slot, d]
            xrs_bf = xp.tile([P, DC * TL], BF16)     # per-tile x row sums (.T)

            xt3 = xt_sb.rearrange("p (t d) -> p t d", t=TL)
            with ExitStack() as str_:
                trps = str_.enter_context(tc.tile_pool(name="trps", bufs=3, space="PSUM"))
                for dc in range(DC):
                    ps = trps.tile([P, B], F32, tag="tr")
                    for tcc in range(TL):
                        nc.tensor.matmul(ps[:, tcc * P:(tcc + 1) * P],
                                         xt3[:, tcc, dc * P:(dc + 1) * P], identbf[:],
                                         start=True, stop=True)
                    (nc.vector.tensor_copy if dc % 2 == 0 else nc.scalar.copy)(
                        xT8[:, dc * B:(dc + 1) * B], ps[:])
                    (nc.scalar.copy if dc % 2 == 0 else nc.vector.tensor_copy)(
                        xTb[:, dc * B:(dc + 1) * B], ps[:])

            pps = s1.enter_context(tc.tile_pool(name="pps", bufs=3, space="PSUM"))
            wk3 = wk_sb.rearrange("p (dc d) -> p dc d", dc=DC)
            wq3 = wq_sb.rearrange("p (dc d) -> p dc d", dc=DC)
            wv3 = wv_sb.rearrange("p (dc d) -> p dc d", dc=DC)
            wvb3 = wvb_sb.rearrange("p (dc d) -> p dc d", dc=DC)
            xT83 = xT8.rearrange("p (dc b) -> p dc b", dc=DC)

            # kT projection (gates the first collectives)
            for dco in range(DC):
                k_ps = pps.tile([P, B], F32, tag="pp", name="k_ps")
                for pp_ in range(DC // 2):
                    nc.tensor.matmul(
                        k_ps[:],
                        wk3[:, 2 * pp_:2 * pp_ + 2, dco * P:(dco + 1) * P],
                        xT83[:, 2 * pp_:2 * pp_ + 2, :],
                        start=(pp_ == 0), stop=(pp_ == DC // 2 - 1),
                        perf_mode=DR,
                    )
                nc.vector.tensor_copy(kT_loc[:, dco * B:(dco + 1) * B], k_ps[:])
            kT3 = kT_loc.rearrange("p (dc b) -> p dc b", dc=DC)
            for h, cc_h in enumerate([cc1a_in, cc1b_in]):
                nc.sync.dma_start(
                    cc_h[:].rearrange("(dc p i) -> p dc i", dc=DC, p=P),
                    kT3[:, :, h * B2:(h + 1) * B2],
                )
                nc.gpsimd.collective_compute(
                    "AllGather", ALU.bypass,
                    replica_groups=[list(range(n_cores))],
                    ins=[cc_h.opt()], outs=[[cc1a_out, cc1b_out][h].opt()],
                )

            # qT projection
            for dco in range(DC):
                q_ps = pps.tile([P, B], F32, tag="pp", name="q_ps")
                for pp_ in range(DC // 2):
                    nc.tensor.matmul(
                        q_ps[:],
                        wq3[:, 2 * pp_:2 * pp_ + 2, dco * P:(dco + 1) * P],
                        xT83[:, 2 * pp_:2 * pp_ + 2, :],
                        start=(pp_ == 0), stop=(pp_ == DC // 2 - 1),
                        perf_mode=DR,
                    )
                nc.vector.tensor_copy(qT[:, dco * B:(dco + 1) * B], q_ps[:])

            # per-tile x row sums (bf16, for the high-precision VS aggregates)
            xrs_f = xp.tile([P, DC * TL], F32)
            for dc in range(DC):
                nc.vector.reduce_sum(
                    xrs_f[:, dc * TL:(dc + 1) * TL],
                    xTb[:, dc * B:(dc + 1) * B].rearrange("p (t i) -> p t i", t=TL),
                    axis=AX.X,
                )
            nc.vector.tensor_copy(xrs_bf[:], xrs_f[:])

            # V projection, fp8 DR; v full scale (wv prescaled x8, cast /8)
            vp4 = vpair.rearrange("p (pr s d) -> p pr s d", pr=2, s=2)
            for tcc in range(TL):
                for g in range(2):
                    v_ps = pps.tile([P, D // 2], F32, tag="pp", name="v_ps")
                    for pp_ in range(DC // 2):
                        nc.tensor.matmul(
                            v_ps[:],
                            xT83[:, 2 * pp_:2 * pp_ + 2, tcc * P:(tcc + 1) * P],
                            wv3[:, 2 * pp_:2 * pp_ + 2, g * (D // 2):(g + 1) * (D // 2)],
                            start=(pp_ == 0), stop=(pp_ == DC // 2 - 1),
                            perf_mode=DR,
                        )
                    nc.vector.tensor_scalar(
                        vp4[:, tcc // 2, tcc % 2, g * (D // 2):(g + 1) * (D // 2)],
                        v_ps[:], 1.0 / 8.0, None, op0=ALU.mult)

            # VS aggregates: VS[t] = xrs[t] @ wv_bf  ([TL, D], bf16 path)
            vs_loc = xp.tile([TL, D], F32)
            for g in range(2):
                dsl = slice(g * (D // 2), (g + 1) * (D // 2))
                vs_ps = pps.tile([TL, D // 2], F32, tag="pp", name=f"vs_ps{g}")
                for dc in range(DC):
                    nc.tensor.matmul(
                        vs_ps[:], xrs_bf[:, dc * TL:(dc + 1) * TL], wvb3[:, dc, dsl],
                        start=(dc == 0), stop=(dc == DC - 1),
                    )
                nc.vector.tensor_copy(vs_loc[:, dsl], vs_ps[:])

            # PVt: within-tile prefix sums of v (fp8 in/out, f32 psum)
            pv_loc = xp.tile([P, TL * D], FP8)       # [p, tile, d]
            pv3 = pv_loc.rearrange("p (t d) -> p t d", t=TL)
            with ExitStack() as spv:
                pvtp = spv.enter_context(tc.tile_pool(name="pvtp", bufs=4, space="PSUM"))
                for tcc in range(TL):
                    for g in range(2):
                        dsl = slice(g * (D // 2), (g + 1) * (D // 2))
                        pv_ps = pvtp.tile([P, D // 2], F32, tag="pv")
                        nc.tensor.matmul(
                            pv_ps[:], triu8_sb[:], vp4[:, tcc // 2, tcc % 2, dsl],
                            start=True, stop=True,
                        )
                        nc.vector.tensor_copy(pv3[:, tcc, dsl], pv_ps[:])

            nc.sync.dma_start(
                cc2_in[0:PVN].rearrange("(t p d) -> p t d", t=TL, p=P),
                pv3.bitcast(U8),
            )
            nc.sync.dma_start(
                cc2_in[PVN:].rearrange("(t d) -> t d", t=TL),
                vs_loc[:].bitcast(U8),
            )

        nc.gpsimd.collective_compute(
            "AllGather", ALU.bypass,
            replica_groups=[list(range(n_cores))],
            ins=[cc2_in.opt()], outs=[cc2_out.opt()],
        )

        # ------------------- phase A: scores / exp / selector -------------------
        m04 = m0.rearrange("p (pr s b) -> p pr s b", pr=NPAIR, s=2)
        qT3 = qT.rearrange("p (dc b) -> p dc b", dc=DC)
        selw4 = selw_sb.rearrange("p (pr s c) -> p pr s c", pr=NPAIR, s=2)
        with ExitStack() as pa:
            ktp = pa.enter_context(tc.tile_pool(name="kt", bufs=4))
            ztp = pa.enter_context(tc.tile_pool(name="zt", bufs=3, space="PSUM"))
            rdp = pa.enter_context(tc.tile_pool(name="rd", bufs=1, space="PSUM"))
            rd_ps = rdp.tile([P, B], F32)

            for h in range(2):
                cc_h = [cc1a_out, cc1b_out][h]
                for rc in range(n_cores):
                    pair = rc * 2 + h
                    ktc = ktp.tile([P, DC * B2], FP8, tag="kt")
                    nc.sync.dma_start(
                        ktc.rearrange("p (dc i) -> p dc i", dc=DC),
                        cc_h[rc, :].rearrange("(dc p i) -> p dc i", dc=DC, p=P),
                    )
                    ktc3 = ktc.rearrange("p (dc i) -> p dc i", dc=DC)
                    for s_ in range(2):
                        zt = ztp.tile([P, B], F32, tag="zt")
                        for pp in range(DC // 2):
                            nc.tensor.matmul(
                                zt[:],
                                ktc3[:, 2 * pp:2 * pp + 2, s_ * P:(s_ + 1) * P],
                                qT3[:, 2 * pp:2 * pp + 2, :],
                                start=(pp == 0), stop=(pp == DC // 2 - 1),
                                perf_mode=DR,
                            )
                        nc.scalar.activation(
                            m04[:, pair, s_, :], zt[:], AF.Exp,
                            bias=expb[:], scale=scale)
                    first = (h == 0 and rc == 0)
                    last = (h == 1 and rc == n_cores - 1)
                    nc.tensor.matmul(
                        rd_ps[:], selw4[:, pair, :, :], m04[:, pair, :, :],
                        start=first, stop=last, perf_mode=DR,
                    )

            # rs (per-tile rowsums) + den out of the selector psum
            nc.vector.tensor_copy(rs_sb[:], rd_ps[0:T, :])
            nc.vector.memset(den_pad[:], 0.0)
            nc.vector.tensor_copy(den_pad[64:65, :], rd_ps[64:65, :])
            for qc in range(QC):
                dps = ztp.tile([P, P], F32, tag="zt")
                nc.tensor.transpose(dps[:], den_pad[:, qc * P:(qc + 1) * P], ident_sb[:])
                nc.vector.tensor_copy(dennat[:, qc:qc + 1], dps[:, 64:65])
            nc.vector.reciprocal(recip[:], dennat[:])

            # CVS = stril32 @ VS_all (bf16), needs gather 2
            for rc in range(n_cores):
                nc.sync.dma_start(
                    vs_all[rc * TL:(rc + 1) * TL, :].bitcast(U8),
                    cc2_out[rc, PVN:].rearrange("(t d) -> t d", t=TL),
                )
            nc.vector.tensor_copy(vs_bf[:], vs_all[:])
            for g in range(2):
                cvs_ps = rdp.tile([T, D // 2], F32, tag="rd2", name=f"cvs_ps{g}")
                nc.tensor.matmul(cvs_ps[:], stril32T_sb[:],
                                 vs_bf[:, g * 512:(g + 1) * 512],
                                 start=True, stop=True)
                nc.vector.tensor_copy(cvs_sb[:, g * 512:(g + 1) * 512], cvs_ps[:])

        # gathered PV tiles -> SBUF (one DMA per remote core)
        for rc in range(n_cores):
            nc.sync.dma_start(
                pvg.rearrange("p (t d) -> p t d", t=T)[:, rc * TL:(rc + 1) * TL, :].bitcast(U8),
                cc2_out[rc, 0:PVN].rearrange("(t p d) -> p t d", t=TL, p=P),
            )

        # ------------------- phase B: num accumulation -------------------
        pvg4 = pvg.rearrange("p (pr s d) -> p pr s d", pr=NPAIR, s=2)
        with ExitStack() as pb:
            nump = pb.enter_context(tc.tile_pool(name="nump", bufs=8, space="PSUM"))
            osb = pb.enter_context(tc.tile_pool(name="osb", bufs=4))
            for g in range(2):
                dsl = slice(g * 512, (g + 1) * 512)
                nums = [nump.tile([P, 512], F32, tag="num", name=f"num{g}_{qc}")
                        for qc in range(QC)]
                for pair in range(NPAIR):
                    for qc in range(QC):
                        nc.tensor.matmul(
                            nums[qc][:],
                            m04[:, pair, :, qc * P:(qc + 1) * P],
                            pvg4[:, pair, :, dsl],
                            start=(pair == 0), stop=False,
                            perf_mode=DR,
                        )
                for qc in range(QC):
                    nc.tensor.matmul(
                        nums[qc][:],
                        rs_sb[:, qc * P:(qc + 1) * P], cvs_sb[:, dsl],
                        start=False, stop=True,
                    )
                for qc in range(QC):
                    ot = osb.tile([P, 512], F32, tag="ot")
                    nc.vector.tensor_scalar(
                        ot[:], nums[qc][:], recip[:, qc:qc + 1], 1.0 / 64.0,
                        op0=ALU.mult, op1=ALU.mult)
                    nc.scalar.dma_start(out.ap()[qc * P:(qc + 1) * P, dsl], ot[:])

    nc.compile()
    return nc


def make_in_maps(x_full, wq, wk, wv, n_cores=8):
    import ml_dtypes
    bf = lambda a: np.ascontiguousarray(a).astype(ml_dtypes.bfloat16)
    f8 = lambda a: np.ascontiguousarray(a).astype(ml_dtypes.float8_e4m3)
    SEQ, D = x_full.shape
    DC = D // P
    B = SEQ // n_cores
    consts = make_consts(SEQ, n_cores)
    # weight images: [p, dc*D + j] = w[dc*128 + p, j]; fp8 ones prescaled x8
    img8 = lambda w: f8((w * 8.0).reshape(DC, P, D).transpose(1, 0, 2).reshape(P, DC * D))
    imgb = lambda w: bf(w.reshape(DC, P, D).transpose(1, 0, 2).reshape(P, DC * D))
    in_maps = []
    for c in range(n_cores):
        m = {"x": bf(x_full[c * B:(c + 1) * B]),
             "wq": img8(wq), "wk": img8(wk), "wv": img8(wv), "wvb": imgb(wv)}
        m.update(consts)
        in_maps.append(m)
    return in_maps


def algo_ref(x, wq, wk, wv):
    """Numpy float64 reference of the restructured math (for validation)."""
    x = x.astype(np.float64)
    q = x @ wq.astype(np.float64)
    k = x @ wk.astype(np.float64)
    v = x @ wv.astype(np.float64)
    z = q @ k.T / np.sqrt(k.shape[1])
    s = np.exp(z)
    pv = np.cumsum(v, axis=0)
    num = s @ pv
    den = s @ (np.arange(1, x.shape[0] + 1, dtype=np.float64))
    return (num / den[:, None]).astype(np.float32)


# ----------------------------------------------------------------------------
# Harness entry point: full (unsharded) inputs -> full output.
# ----------------------------------------------------------------------------
SEQ, D_IN, N_CORES = 4096, 1024, 8
_built = {}


def _get_nc(SEQ_=SEQ, D_=D_IN, n_cores=N_CORES):
    key = (SEQ_, D_, n_cores)
    if key not in _built:
        _built[key] = build(SEQ=SEQ_, D=D_, n_cores=n_cores)
    return _built[key]


def run(x, wq, wk, wv, trace=False, **spmd_kwargs):
    from concourse.bass_utils import run_bass_kernel_spmd

    x = np.ascontiguousarray(np.asarray(x, dtype=np.float32))
    wq = np.ascontiguousarray(np.asarray(wq, dtype=np.float32))
    wk = np.ascontiguousarray(np.asarray(wk, dtype=np.float32))
    wv = np.ascontiguousarray(np.asarray(wv, dtype=np.float32))
    n_cores = N_CORES
    nc = _get_nc(x.shape[0], x.shape[1], n_cores)
    in_maps = make_in_maps(x, wq, wk, wv, n_cores=n_cores)
    res = run_bass_kernel_spmd(nc, in_maps, list(range(n_cores)),
                               trace=trace, **spmd_kwargs)
    out = np.concatenate([res.results[c]["out"] for c in range(n_cores)], axis=0)
    return out, res


def kernel(x, wq, wk, wv):
    out, _ = run(x, wq, wk, wv, trace=False)
    return out
